# revision 15
# baseline (speedup 1.0000x reference)
"""Trainium2 Bass kernel for StyleGAN2-style upsampled Conv1d.

Reference computation (x:(16,256,4096), weight:(256,256,3), bias:(256,)):
  y = conv_transpose1d(x, weight, stride=2)      # correlation on 2x-dilated x
  z = upfirdn1d(y, [1,3,3,1]/8 * 2)              # depthwise FIR
  out = z + bias                                  # (16, 256, 8192)

The transposed conv + FIR collapse into TWO 3-tap correlations over the
original x grid (even/odd output phases) with folded tap matrices:
  A  = .75 w0 + .25 w1   B  = .25 w0 + .75 w1 + .75 w2   C  = .25 w2
  A' = .25 w0            B' = .75 w0 + .75 w1 + .25 w2   C' = .25 w1 + .75 w2

PE strategy (trace-measured: every 512-col matmul issues at ~218 ns
regardless of dtype; fp8 DoubleRow contracts 256 deep per column, bf16
128): mixed-dtype accumulation chains.  The two big taps of each phase
run as exact bf16 matmuls (2 k-tiles each); the small .25-coefficient
tap (C / A') runs as ONE fp8e4m3 DoubleRow matmul.  That is 5 matmuls
per 512-position chain vs 6 all-bf16, i.e. 10 per chunk-pair vs the 12
of the fp32r baseline (~17% fewer PE columns).  All operands carry a
shared 2^11 scale (x*16, W*128, exact in bf16) so both dtypes accumulate
into one PSUM bank; drains fold the 1/2048 descale and bias into one op.
Small-tap fp8 rounding gives rel err ~7e-3 vs the 2e-2 gate.

Pipelining: per-chunk PSUM banks (8 x [128,512] in flight, no weight-
stationary batching -- LDWEIGHTS is fully hidden), drains alternate
vector/scalar, output DMAs rotate over the sync/scalar/gpsimd queues.
Inputs are host-packed k-interleaved so each x tile loads as one
contiguous-per-partition DMA.  Sharding: batch-parallel, 2 per core.
"""

import numpy as np
import ml_dtypes

import concourse.bass as bass
import concourse.mybir as mybir
import concourse.tile as tile
from concourse import bacc
from concourse.bass_utils import run_bass_kernel_spmd

N, IN_CH, OUT_CH, KERNEL, D = 16, 256, 256, 3, 4096
NCORES = 8
BPC = N // NCORES          # batches per core
DOUT = 2 * D
F32 = mybir.dt.float32
BF16 = mybir.dt.bfloat16
FP8 = mybir.dt.float8e4
E4M3 = ml_dtypes.float8_e4m3
NPBF16 = ml_dtypes.bfloat16

SX = 16.0                  # x scale (power of 2, shared by bf16 + fp8 copies)
SW = 128.0                 # weight scale (power of 2)
INV_S = 1.0 / (SX * SW)

NCHUNK = 512               # matmul moving free dim (= one PSUM bank of fp32)
NCHUNKS = D // NCHUNK      # 8
NWARM = 7                  # PE clock warm-up matmuls
DP2 = D + 2

# per-phase tap plan: True = big tap (exact bf16), False = small (fp8 DR)
TAP_BIG = [
    [True, True, False],   # even phase: A, B big; C = .25 w2 small
    [False, True, True],   # odd phase: A' = .25 w0 small; B', C' big
]

_CACHED = {}


def _bblk(phase, tap, k, m):
    # bf16 blocks [128 K, 128 M] for big taps, indexed densely
    order = []
    for ph in range(2):
        for t in range(3):
            if TAP_BIG[ph][t]:
                order.append((ph, t))
    idx = order.index((phase, tap))
    return m * 8 + idx * 2 + k


def _sblk(phase, m):
    # fp8 DR blocks [128 K, 2 slots, 128 M] for the small tap of `phase`
    return phase * 2 + m


def _build_nc():
    nc = bacc.Bacc("TRN2", target_bir_lowering=False, debug=False)

    # x arrives host-scaled by SX, padded with zero columns at 0 and D+1,
    # k-interleaved ([128 part, 2 k, D+2] flattened) in bf16 + e4m3 copies.
    xb_t = nc.dram_tensor("xb", [BPC, 128, 2 * DP2], BF16, kind="ExternalInput")
    wb_t = nc.dram_tensor("wb", [128, 16 * 128], BF16, kind="ExternalInput")
    w8_t = nc.dram_tensor("w8", [128, 4 * 256], FP8, kind="ExternalInput")
    b_t = nc.dram_tensor("b", [128, 2], F32, kind="ExternalInput")
    o_t = nc.dram_tensor("out", [BPC, OUT_CH, DOUT], F32, kind="ExternalOutput")

    with tile.TileContext(nc) as tc:
        with (
            tc.tile_pool(name="wpool", bufs=1) as wpool,
            tc.tile_pool(name="xpool", bufs=2 * BPC) as xpool,
            tc.tile_pool(name="zpool", bufs=8) as zpool,
            tc.tile_pool(name="ppool", bufs=8, space="PSUM") as ppool,
        ):
            # weights + bias first on the sync HWDGE queue (m0 half only;
            # the m1 half follows bb0's x blocks -- needed ~17us later)
            wb_sb = wpool.tile([128, 16 * 128], BF16)
            nc.sync.dma_start(out=wb_sb[:, 0:8 * 128], in_=wb_t[:, 0:8 * 128])
            w8_sb = wpool.tile([128, 4 * 256], FP8)
            nc.sync.dma_start(out=w8_sb[:], in_=w8_t[:])
            b_sb = wpool.tile([128, 2], F32)
            nc.sync.dma_start(out=b_sb[:], in_=b_t[:])

            xb_sb, x8_sb = {}, {}
            for bb in range(BPC):
                xb_sb[bb] = xpool.tile([128, 2, DP2], BF16, tag="x", name=f"xb_{bb}")
                x8_sb[bb] = xpool.tile([128, 2, DP2], FP8, tag="x", name=f"x8_{bb}")
            # First-needed blocks first, per queue (queues are FIFO);
            # input staging is DMA-fabric-bound early, so bb0 streams in
            # chunk-consumption order in fine blocks:
            #   sync:    w, b, then bf16 bb0 in 4 column blocks (both k each)
            #   scalar:  fp8 bb0 cols [0, q1) (needed by c0's DR matmul),
            #            then bf16 bb1 (whole)
            #   gpsimd:  fp8 bb0 cols [q1, D+2), fp8 bb1 (SWDGE)
            q1 = 2 * NCHUNK + 3
            edges = [0, 515, 1027, 2051, 3075, DP2]
            for lo, hi in zip(edges[:-1], edges[1:]):
                nc.sync.dma_start(
                    out=xb_sb[0][:, 0, lo:hi],
                    in_=xb_t[0, :, lo:hi],
                )
                nc.scalar.dma_start(
                    out=xb_sb[0][:, 1, lo:hi],
                    in_=xb_t[0, :, DP2 + lo:DP2 + hi],
                )
            # m1 weights + bb1 x ride the TAIL of the sync queue: they enter
            # the DMA fabric only after all of bb0 is resident.
            nc.sync.dma_start(out=wb_sb[:, 8 * 128:], in_=wb_t[:, 8 * 128:])
            nc.sync.dma_start(
                out=xb_sb[1][:].rearrange("p two d -> p (two d)"),
                in_=xb_t[1],
            )
            # fp8 x copies are CAST ON-CHIP from the bf16 tiles (saves 2.1MB
            # of early DMA-fabric bytes): c0's block first on scalar (needed
            # by c0's DR matmul), rest on vector; bb1 mid-loop below.
            nc.scalar.activation(
                out=x8_sb[0][:, :, 0:515], in_=xb_sb[0][:, :, 0:515],
                func=mybir.ActivationFunctionType.Copy,
            )
            nc.scalar.activation(
                out=x8_sb[0][:, :, 515:q1], in_=xb_sb[0][:, :, 515:q1],
                func=mybir.ActivationFunctionType.Copy,
            )
            nc.vector.tensor_scalar(
                out=x8_sb[0][:, :, q1:DP2], in0=xb_sb[0][:, :, q1:DP2],
                scalar1=0.0, scalar2=None, op0=mybir.AluOpType.add,
            )

            # Pre-warm the PE while inputs load: dummy bf16 matmuls on a
            # memset tile ramp the PE clock before the real work arrives.
            warm_bf = wpool.tile([128, 128 + NCHUNK], BF16)
            nc.vector.memset(warm_bf[:], 1.0)
            warm_ps = ppool.tile([128, NCHUNK], F32, tag="bank", name="warm_ps")
            for _ in range(NWARM):
                nc.tensor.matmul(
                    warm_ps[:],
                    lhsT=warm_bf[:, 0:128],
                    rhs=warm_bf[:, 128:128 + NCHUNK],
                    start=True,
                    stop=True,
                )

            OUT_Q = [nc.scalar, nc.gpsimd, nc.sync]
            qi = 0
            for bb in range(BPC):
                for m in range(2):
                    bias_ap = b_sb[:, m:m + 1]
                    for c in range(NCHUNKS):
                        if bb == 0 and m == 0 and c == 2:
                            nc.vector.tensor_scalar(
                                out=x8_sb[1][:], in0=xb_sb[1][:],
                                scalar1=0.0, scalar2=None,
                                op0=mybir.AluOpType.add,
                            )
                        banks = []
                        for phase in range(2):
                            ps = ppool.tile([128, NCHUNK], F32, tag="bank",
                                            name=f"ps_{bb}_{m}_{c}_{phase}")
                            banks.append(ps)
                            si, nst = 0, 5
                            # small (fp8 DR) tap LAST in every chain: a DR
                            # matmul issues ~28ns after a bf16 but ~404ns
                            # after another DR, so keep DRs separated
                            taporder = [t for t in range(3) if TAP_BIG[phase][t]]
                            taporder += [t for t in range(3) if not TAP_BIG[phase][t]]
                            for tap in taporder:
                                w0 = NCHUNK * c + tap
                                if TAP_BIG[phase][tap]:
                                    for k in range(2):
                                        blk = _bblk(phase, tap, k, m)
                                        nc.tensor.matmul(
                                            ps[:],
                                            lhsT=wb_sb[:, blk * 128:(blk + 1) * 128],
                                            rhs=xb_sb[bb][:, k, w0:w0 + NCHUNK],
                                            start=(si == 0),
                                            stop=(si == nst - 1),
                                        )
                                        si += 1
                                else:
                                    blk = _sblk(phase, m)
                                    nc.tensor.matmul(
                                        ps[:],
                                        lhsT=w8_sb[:, blk * 256:(blk + 1) * 256]
                                        .rearrange("p (two m) -> p two m", two=2),
                                        rhs=x8_sb[bb][:, :, w0:w0 + NCHUNK],
                                        start=(si == 0),
                                        stop=(si == nst - 1),
                                        perf_mode=mybir.MatmulPerfMode.DoubleRow,
                                    )
                                    si += 1
                        zt = zpool.tile([128, 2 * NCHUNK], F32, tag="z",
                                        name=f"z_{bb}_{m}_{c}")
                        zv = zt[:].rearrange("p (j two) -> p two j", two=2)
                        # one drain per phase bank: descale + bias, writing
                        # stride-2 interleaved into the final layout
                        nc.vector.tensor_scalar(
                            out=zv[:, 0, :], in0=banks[0][:],
                            scalar1=INV_S, scalar2=bias_ap,
                            op0=mybir.AluOpType.mult,
                            op1=mybir.AluOpType.add,
                        )
                        nc.scalar.activation(
                            out=zv[:, 1, :], in_=banks[1][:],
                            func=mybir.ActivationFunctionType.Identity,
                            bias=bias_ap, scale=INV_S,
                        )
                        if qi >= 29:
                            OUT_Q[qi % 3].dma_start(
                                out=o_t[bb, m * 128:(m + 1) * 128,
                                        c * 2 * NCHUNK:c * 2 * NCHUNK + NCHUNK],
                                in_=zt[:, 0:NCHUNK],
                            )
                            OUT_Q[(qi + 1) % 3].dma_start(
                                out=o_t[bb, m * 128:(m + 1) * 128,
                                        c * 2 * NCHUNK + NCHUNK:(c + 1) * 2 * NCHUNK],
                                in_=zt[:, NCHUNK:],
                            )
                        else:
                            OUT_Q[qi % 3].dma_start(
                                out=o_t[bb, m * 128:(m + 1) * 128,
                                        c * 2 * NCHUNK:(c + 1) * 2 * NCHUNK],
                                in_=zt[:],
                            )
                        qi += 1
    nc.compile()
    return nc


def _host_weights(weight, bias):
    w = np.asarray(weight, dtype=np.float32)
    w0, w1, w2 = w[:, :, 0], w[:, :, 1], w[:, :, 2]
    taps = [
        [0.75 * w0 + 0.25 * w1, 0.25 * w0 + 0.75 * w1 + 0.75 * w2, 0.25 * w2],
        [0.25 * w0, 0.75 * w0 + 0.75 * w1 + 0.25 * w2, 0.25 * w1 + 0.75 * w2],
    ]
    wb_host = np.zeros((128, 16 * 128), dtype=NPBF16)
    w8_host = np.zeros((128, 4 * 256), dtype=E4M3)
    for phase in range(2):
        for tap in range(3):
            full = taps[phase][tap] * SW  # [256 out, 256 in]
            if TAP_BIG[phase][tap]:
                for k in range(2):
                    for m in range(2):
                        blk = _bblk(phase, tap, k, m)
                        # lhsT block[i, o] = full[m*128+o, k*128+i]
                        sub = full[m * 128:(m + 1) * 128, k * 128:(k + 1) * 128]
                        wb_host[:, blk * 128:(blk + 1) * 128] = sub.T.astype(NPBF16)
            else:
                for m in range(2):
                    blk = _sblk(phase, m)
                    # DR block[p, slot i, o] = full[m*128+o, i*128+p]
                    sub = full[m * 128:(m + 1) * 128, :]  # [128 M, 256 K]
                    arr = sub.reshape(128, 2, 128).transpose(2, 1, 0)
                    w8_host[:, blk * 256:(blk + 1) * 256] = (
                        arr.reshape(128, 256).astype(E4M3)
                    )
    b_host = np.asarray(bias, dtype=np.float32).reshape(2, 128).T.copy()
    return wb_host, w8_host, b_host


def _host_x(x):
    x = np.asarray(x, dtype=np.float32)
    xp = np.pad(x, ((0, 0), (0, 0), (1, 1))) * SX    # [N, 256, D+2]
    # k-interleave: [N, 2, 128, D+2] -> [N, 128, 2*(D+2)]
    xi = xp.reshape(x.shape[0], 2, 128, DP2).transpose(0, 2, 1, 3)
    xi = np.ascontiguousarray(xi).reshape(x.shape[0], 128, 2 * DP2)
    return xi.astype(NPBF16)


def make_in_maps(x, weight, bias):
    xb = _host_x(x)
    wb_host, w8_host, b_host = _host_weights(weight, bias)
    in_maps = []
    for core in range(NCORES):
        sl = slice(core * BPC, (core + 1) * BPC)
        in_maps.append({
            "xb": np.ascontiguousarray(xb[sl]),
            "wb": wb_host,
            "w8": w8_host,
            "b": b_host,
        })
    return in_maps


def kernel(x, weight, bias):
    if "nc" not in _CACHED:
        _CACHED["nc"] = _build_nc()
    nc = _CACHED["nc"]
    in_maps = make_in_maps(x, weight, bias)
    res = run_bass_kernel_spmd(nc, in_maps, core_ids=list(range(NCORES)))
    out = np.concatenate([np.asarray(r["out"]) for r in res.results], axis=0)
    return out


# revision 16
# speedup vs baseline: 1.0216x; 1.0216x over previous
"""Trainium2 Bass kernel for StyleGAN2-style upsampled Conv1d.

Reference computation (x:(16,256,4096), weight:(256,256,3), bias:(256,)):
  y = conv_transpose1d(x, weight, stride=2)      # correlation on 2x-dilated x
  z = upfirdn1d(y, [1,3,3,1]/8 * 2)              # depthwise FIR
  out = z + bias                                  # (16, 256, 8192)

The transposed conv + FIR collapse into TWO 3-tap correlations over the
original x grid (even/odd output phases) with folded tap matrices:
  A  = .75 w0 + .25 w1   B  = .25 w0 + .75 w1 + .75 w2   C  = .25 w2
  A' = .25 w0            B' = .75 w0 + .75 w1 + .25 w2   C' = .25 w1 + .75 w2

PE strategy (trace-measured: every 512-col matmul issues at ~218 ns
regardless of dtype; fp8 DoubleRow contracts 256 deep per column, bf16
128): mixed-dtype accumulation chains.  The two big taps of each phase
run as exact bf16 matmuls (2 k-tiles each); the small .25-coefficient
tap (C / A') runs as ONE fp8e4m3 DoubleRow matmul.  That is 5 matmuls
per 512-position chain vs 6 all-bf16, i.e. 10 per chunk-pair vs the 12
of the fp32r baseline (~17% fewer PE columns).  All operands carry a
shared 2^11 scale (x*16, W*128, exact in bf16) so both dtypes accumulate
into one PSUM bank; drains fold the 1/2048 descale and bias into one op.
Small-tap fp8 rounding gives rel err ~7e-3 vs the 2e-2 gate.

Pipelining: per-chunk PSUM banks (8 x [128,512] in flight, no weight-
stationary batching -- LDWEIGHTS is fully hidden), drains alternate
vector/scalar, output DMAs rotate over the sync/scalar/gpsimd queues.
Inputs are host-packed k-interleaved so each x tile loads as one
contiguous-per-partition DMA.  Sharding: batch-parallel, 2 per core.
"""

import numpy as np
import ml_dtypes

import concourse.bass as bass
import concourse.mybir as mybir
import concourse.tile as tile
from concourse import bacc
from concourse.bass_utils import run_bass_kernel_spmd

N, IN_CH, OUT_CH, KERNEL, D = 16, 256, 256, 3, 4096
NCORES = 8
BPC = N // NCORES          # batches per core
DOUT = 2 * D
F32 = mybir.dt.float32
BF16 = mybir.dt.bfloat16
FP8 = mybir.dt.float8e4
E4M3 = ml_dtypes.float8_e4m3
NPBF16 = ml_dtypes.bfloat16

SX = 16.0                  # x scale (power of 2, shared by bf16 + fp8 copies)
SW = 128.0                 # weight scale (power of 2)
INV_S = 1.0 / (SX * SW)

NCHUNK = 512               # matmul moving free dim (= one PSUM bank of fp32)
NCHUNKS = D // NCHUNK      # 8
NWARM = 7                  # PE clock warm-up matmuls
DP2 = D + 2

# per-phase tap plan: True = big tap (exact bf16), False = small (fp8 DR)
TAP_BIG = [
    [True, True, False],   # even phase: A, B big; C = .25 w2 small
    [False, True, True],   # odd phase: A' = .25 w0 small; B', C' big
]

_CACHED = {}


def _bblk(phase, tap, k, m):
    # bf16 blocks [128 K, 128 M] for big taps, indexed densely
    order = []
    for ph in range(2):
        for t in range(3):
            if TAP_BIG[ph][t]:
                order.append((ph, t))
    idx = order.index((phase, tap))
    return m * 8 + idx * 2 + k


def _sblk(phase, m):
    # fp8 DR blocks [128 K, 2 slots, 128 M] for the small tap of `phase`
    return phase * 2 + m


def _build_nc():
    nc = bacc.Bacc("TRN2", target_bir_lowering=False, debug=False)

    # x arrives host-scaled by SX, padded with zero columns at 0 and D+1,
    # k-interleaved ([128 part, 2 k, D+2] flattened) in bf16 + e4m3 copies.
    xb_t = nc.dram_tensor("xb", [BPC, 128, 2 * DP2], BF16, kind="ExternalInput")
    wb_t = nc.dram_tensor("wb", [128, 16 * 128], BF16, kind="ExternalInput")
    w8_t = nc.dram_tensor("w8", [128, 4 * 256], FP8, kind="ExternalInput")
    b_t = nc.dram_tensor("b", [128, 2], F32, kind="ExternalInput")
    o_t = nc.dram_tensor("out", [BPC, OUT_CH, DOUT], F32, kind="ExternalOutput")

    with tile.TileContext(nc) as tc:
        with (
            tc.tile_pool(name="wpool", bufs=1) as wpool,
            tc.tile_pool(name="xpool", bufs=2 * BPC) as xpool,
            tc.tile_pool(name="zpool", bufs=8) as zpool,
            tc.tile_pool(name="ppool", bufs=8, space="PSUM") as ppool,
        ):
            # weights + bias first on the sync HWDGE queue (m0 half only;
            # the m1 half follows bb0's x blocks -- needed ~17us later)
            wb_sb = wpool.tile([128, 16 * 128], BF16)
            nc.sync.dma_start(out=wb_sb[:, 0:8 * 128], in_=wb_t[:, 0:8 * 128])
            w8_sb = wpool.tile([128, 4 * 256], FP8)
            nc.sync.dma_start(out=w8_sb[:], in_=w8_t[:])
            b_sb = wpool.tile([128, 2], F32)
            nc.sync.dma_start(out=b_sb[:], in_=b_t[:])

            xb_sb, x8_sb = {}, {}
            for bb in range(BPC):
                xb_sb[bb] = xpool.tile([128, 2, DP2], BF16, tag="x", name=f"xb_{bb}")
                x8_sb[bb] = xpool.tile([128, 2, DP2], FP8, tag="x", name=f"x8_{bb}")
            # First-needed blocks first, per queue (queues are FIFO);
            # input staging is DMA-fabric-bound early, so bb0 streams in
            # chunk-consumption order in fine blocks:
            #   sync:    w, b, then bf16 bb0 in 4 column blocks (both k each)
            #   scalar:  fp8 bb0 cols [0, q1) (needed by c0's DR matmul),
            #            then bf16 bb1 (whole)
            #   gpsimd:  fp8 bb0 cols [q1, D+2), fp8 bb1 (SWDGE)
            q1 = 2 * NCHUNK + 3
            edges = [0, 515, 1027, 2051, 3075, DP2]
            for lo, hi in zip(edges[:-1], edges[1:]):
                nc.sync.dma_start(
                    out=xb_sb[0][:, 0, lo:hi],
                    in_=xb_t[0, :, lo:hi],
                )
                nc.scalar.dma_start(
                    out=xb_sb[0][:, 1, lo:hi],
                    in_=xb_t[0, :, DP2 + lo:DP2 + hi],
                )
            # m1 weights + bb1 x ride the TAIL of the sync queue: they enter
            # the DMA fabric only after all of bb0 is resident.
            nc.sync.dma_start(out=wb_sb[:, 8 * 128:], in_=wb_t[:, 8 * 128:])
            nc.sync.dma_start(
                out=xb_sb[1][:].rearrange("p two d -> p (two d)"),
                in_=xb_t[1],
            )
            # fp8 x copies are CAST ON-CHIP from the bf16 tiles (saves 2.1MB
            # of early DMA-fabric bytes): c0's block first on scalar (needed
            # by c0's DR matmul), rest on vector; bb1 mid-loop below.
            nc.scalar.activation(
                out=x8_sb[0][:, :, 0:515], in_=xb_sb[0][:, :, 0:515],
                func=mybir.ActivationFunctionType.Copy,
            )
            nc.scalar.activation(
                out=x8_sb[0][:, :, 515:q1], in_=xb_sb[0][:, :, 515:q1],
                func=mybir.ActivationFunctionType.Copy,
            )
            nc.vector.tensor_scalar(
                out=x8_sb[0][:, :, q1:DP2], in0=xb_sb[0][:, :, q1:DP2],
                scalar1=0.0, scalar2=None, op0=mybir.AluOpType.add,
            )

            # Pre-warm the PE while inputs load: dummy bf16 matmuls on a
            # memset tile ramp the PE clock before the real work arrives.
            warm_bf = wpool.tile([128, 128 + NCHUNK], BF16)
            nc.vector.memset(warm_bf[:], 1.0)
            warm_ps = ppool.tile([128, NCHUNK], F32, tag="bank", name="warm_ps")
            for _ in range(NWARM):
                nc.tensor.matmul(
                    warm_ps[:],
                    lhsT=warm_bf[:, 0:128],
                    rhs=warm_bf[:, 128:128 + NCHUNK],
                    start=True,
                    stop=True,
                )

            OUT_Q = [nc.scalar, nc.gpsimd, nc.sync]
            qi = 0
            for bb in range(BPC):
                for m in range(2):
                    bias_ap = b_sb[:, m:m + 1]
                    for c in range(NCHUNKS):
                        if bb == 0 and m == 0 and c == 2:
                            nc.vector.tensor_scalar(
                                out=x8_sb[1][:], in0=xb_sb[1][:],
                                scalar1=0.0, scalar2=None,
                                op0=mybir.AluOpType.add,
                            )
                        banks = []
                        for phase in range(2):
                            ps = ppool.tile([128, NCHUNK], F32, tag="bank",
                                            name=f"ps_{bb}_{m}_{c}_{phase}")
                            banks.append(ps)
                            si, nst = 0, 5
                            for tap in range(3):
                                w0 = NCHUNK * c + tap
                                if TAP_BIG[phase][tap]:
                                    for k in range(2):
                                        blk = _bblk(phase, tap, k, m)
                                        nc.tensor.matmul(
                                            ps[:],
                                            lhsT=wb_sb[:, blk * 128:(blk + 1) * 128],
                                            rhs=xb_sb[bb][:, k, w0:w0 + NCHUNK],
                                            start=(si == 0),
                                            stop=(si == nst - 1),
                                        )
                                        si += 1
                                else:
                                    blk = _sblk(phase, m)
                                    nc.tensor.matmul(
                                        ps[:],
                                        lhsT=w8_sb[:, blk * 256:(blk + 1) * 256]
                                        .rearrange("p (two m) -> p two m", two=2),
                                        rhs=x8_sb[bb][:, :, w0:w0 + NCHUNK],
                                        start=(si == 0),
                                        stop=(si == nst - 1),
                                        perf_mode=mybir.MatmulPerfMode.DoubleRow,
                                    )
                                    si += 1
                        zt = zpool.tile([128, 2 * NCHUNK], F32, tag="z",
                                        name=f"z_{bb}_{m}_{c}")
                        zv = zt[:].rearrange("p (j two) -> p two j", two=2)
                        # one drain per phase bank: descale + bias, writing
                        # stride-2 interleaved into the final layout
                        nc.vector.tensor_scalar(
                            out=zv[:, 0, :], in0=banks[0][:],
                            scalar1=INV_S, scalar2=bias_ap,
                            op0=mybir.AluOpType.mult,
                            op1=mybir.AluOpType.add,
                        )
                        nc.scalar.activation(
                            out=zv[:, 1, :], in_=banks[1][:],
                            func=mybir.ActivationFunctionType.Identity,
                            bias=bias_ap, scale=INV_S,
                        )
                        if qi >= 29:
                            OUT_Q[qi % 3].dma_start(
                                out=o_t[bb, m * 128:(m + 1) * 128,
                                        c * 2 * NCHUNK:c * 2 * NCHUNK + NCHUNK],
                                in_=zt[:, 0:NCHUNK],
                            )
                            OUT_Q[(qi + 1) % 3].dma_start(
                                out=o_t[bb, m * 128:(m + 1) * 128,
                                        c * 2 * NCHUNK + NCHUNK:(c + 1) * 2 * NCHUNK],
                                in_=zt[:, NCHUNK:],
                            )
                        else:
                            OUT_Q[qi % 3].dma_start(
                                out=o_t[bb, m * 128:(m + 1) * 128,
                                        c * 2 * NCHUNK:(c + 1) * 2 * NCHUNK],
                                in_=zt[:],
                            )
                        qi += 1
    nc.compile()
    return nc


def _host_weights(weight, bias):
    w = np.asarray(weight, dtype=np.float32)
    w0, w1, w2 = w[:, :, 0], w[:, :, 1], w[:, :, 2]
    taps = [
        [0.75 * w0 + 0.25 * w1, 0.25 * w0 + 0.75 * w1 + 0.75 * w2, 0.25 * w2],
        [0.25 * w0, 0.75 * w0 + 0.75 * w1 + 0.25 * w2, 0.25 * w1 + 0.75 * w2],
    ]
    wb_host = np.zeros((128, 16 * 128), dtype=NPBF16)
    w8_host = np.zeros((128, 4 * 256), dtype=E4M3)
    for phase in range(2):
        for tap in range(3):
            full = taps[phase][tap] * SW  # [256 out, 256 in]
            if TAP_BIG[phase][tap]:
                for k in range(2):
                    for m in range(2):
                        blk = _bblk(phase, tap, k, m)
                        # lhsT block[i, o] = full[m*128+o, k*128+i]
                        sub = full[m * 128:(m + 1) * 128, k * 128:(k + 1) * 128]
                        wb_host[:, blk * 128:(blk + 1) * 128] = sub.T.astype(NPBF16)
            else:
                for m in range(2):
                    blk = _sblk(phase, m)
                    # DR block[p, slot i, o] = full[m*128+o, i*128+p]
                    sub = full[m * 128:(m + 1) * 128, :]  # [128 M, 256 K]
                    arr = sub.reshape(128, 2, 128).transpose(2, 1, 0)
                    w8_host[:, blk * 256:(blk + 1) * 256] = (
                        arr.reshape(128, 256).astype(E4M3)
                    )
    b_host = np.asarray(bias, dtype=np.float32).reshape(2, 128).T.copy()
    return wb_host, w8_host, b_host


def _host_x(x):
    x = np.asarray(x, dtype=np.float32)
    xp = np.pad(x, ((0, 0), (0, 0), (1, 1))) * SX    # [N, 256, D+2]
    # k-interleave: [N, 2, 128, D+2] -> [N, 128, 2*(D+2)]
    xi = xp.reshape(x.shape[0], 2, 128, DP2).transpose(0, 2, 1, 3)
    xi = np.ascontiguousarray(xi).reshape(x.shape[0], 128, 2 * DP2)
    return xi.astype(NPBF16)


def make_in_maps(x, weight, bias):
    xb = _host_x(x)
    wb_host, w8_host, b_host = _host_weights(weight, bias)
    in_maps = []
    for core in range(NCORES):
        sl = slice(core * BPC, (core + 1) * BPC)
        in_maps.append({
            "xb": np.ascontiguousarray(xb[sl]),
            "wb": wb_host,
            "w8": w8_host,
            "b": b_host,
        })
    return in_maps


def kernel(x, weight, bias):
    if "nc" not in _CACHED:
        _CACHED["nc"] = _build_nc()
    nc = _CACHED["nc"]
    in_maps = make_in_maps(x, weight, bias)
    res = run_bass_kernel_spmd(nc, in_maps, core_ids=list(range(NCORES)))
    out = np.concatenate([np.asarray(r["out"]) for r in res.results], axis=0)
    return out
